# revision 1
# baseline (speedup 1.0000x reference)
"""Trainium2 Bass kernel for nn_Model_11458972746263 (2-stage Aligner:
InterAlign + SelfAlign with SFU fusion blocks, carried E/B attention state).

Sharding: data-parallel over batch — 8 batch elements -> 8 NeuronCores, one
identical Bass program, per-core input maps, weights replicated.

Per-core dataflow (one batch element, PE matmuls in float32r, which streams at
1 cyc/row for moving dims >= 256, ~4x faster than plain fp32):
  canonical state is TRANSPOSED xT (d, c) so every weight matmul
  out^T = act(W^T @ xT + b) takes W as lhsT *as stored* and bias+activation is
  a fused per-partition ACT op on PSUM evacuation.

float32r plumbing: any tensor CONSUMED by an f32r matmul must be produced
with dtype float32r (the producer rounds on write; plain-f32-bitcast is
rejected by the BIR verifier).  DMA from an f32r DRAM tensor counts.  Engines
reading f32r tiles for non-matmul ops use a zero-cost bitcast back to f32.
walrus also only allows ONE sync wait on self-loading (fp32/f32r) matmuls —
_split_matmul_waits() moves surplus waits onto PE NoOps.

Host-side prep (inside kernel(), plain numpy):
  - U -> U^T per core;  SFU weights folded 4d->3d ([x, f, x*f] basis, exact
    reparametrization of [x, f, x*f, x-f] @ W);
  - weights retiled to contiguous [128,128] blocks for max-BW DMA;
  - masks cast to f32 in broadcast ([128,C]) and per-partition column layouts.
"""

import numpy as np

P = 128


def _split_matmul_waits(nc):
    """This walrus build caps sync waits per lowered instruction struct (the
    self-loading fp32/f32r matmul S3_LW takes only ONE; ACT structs are also
    limited). Move surplus waits of every compute-engine instruction onto
    NoOps inserted just before it on the same engine — engine program order
    makes that equivalent."""
    import concourse.mybir as mybir
    skip = (mybir.InstNoOp, mybir.InstEventSemaphore)
    if hasattr(mybir, "InstDrain"):
        skip = skip + (mybir.InstDrain,)
    n_split = 0
    for f in nc.m.functions:
        for b in f.blocks:
            insts = b.instructions
            if not any(len(i.sync_info.on_wait) > 1 for i in insts
                       if i.sync_info is not None):
                continue
            out = []
            for inst in insts:
                si = inst.sync_info
                if (si is not None and len(si.on_wait) > 1
                        and not isinstance(inst, skip)
                        and not isinstance(inst, mybir.InstDMACopy)):
                    waits = list(si.on_wait)
                    for j, w in enumerate(waits[:-1]):
                        nop = mybir.InstNoOp(
                            name=f"{inst.name}_wsplit{j}",
                            engine=inst.engine, ins=[], outs=[],
                            sync_info=mybir.SyncInfo(on_wait=[w],
                                                     on_update=[]))
                        out.append(nop)
                    inst.sync_info = mybir.SyncInfo(
                        on_wait=[waits[-1]], on_update=list(si.on_update))
                    n_split += 1
                out.append(inst)
            b.instructions = out
    return n_split


def _chunks(n, target=384):
    """Split a free dim into PSUM-bank-sized chunks (<=512 fp32)."""
    if n <= 512:
        return [(0, n)]
    assert n % target == 0
    return [(i * target, target) for i in range(n // target)]


# ================================================================ builder
def build_program(C=768, D=768, Q=96, T=2, gammas_i=(3.0, 3.0),
                  gammas_s=(3.0, 3.0)):
    import concourse.mybir as mybir
    import concourse.tile as tile
    from concourse import bacc

    f32 = mybir.dt.float32
    f32r = mybir.dt.float32r
    AF = mybir.ActivationFunctionType
    AX = mybir.AxisListType
    OP = mybir.AluOpType

    KC = C // P
    KD = D // P
    KF = (3 * D) // P
    assert C % P == 0 and D % P == 0 and Q <= P and C == D

    CCH = _chunks(C)

    nc = bacc.Bacc("TRN2", target_bir_lowering=False, debug=False,
                   enable_asserts=True)

    # ---------------- DRAM I/O (per-core tensors) ----------------
    # f32r inputs: anything DMA'd straight into matmul operands.
    UT_d = nc.dram_tensor("UT", [D, C], f32r, kind="ExternalInput")
    V_d = nc.dram_tensor("Vn", [Q, D], f32r, kind="ExternalInput")
    cmbc_d = nc.dram_tensor("cm_bc", [P, C], f32, kind="ExternalInput")
    qmbc_d = nc.dram_tensor("qm_bc", [P, Q], f32, kind="ExternalInput")
    cmcol_d = nc.dram_tensor("cm_cols", [P, KC], f32, kind="ExternalInput")
    ident_d = nc.dram_tensor("ident", [P, P], f32, kind="ExternalInput")
    diagm_d = nc.dram_tensor("diagm", [P, P], f32, kind="ExternalInput")
    ones_d = nc.dram_tensor("ones_cr", [P, 1], f32, kind="ExternalInput")
    Wu_d = nc.dram_tensor("Wu_t", [T, KD, KD, P, P], f32r, kind="ExternalInput")
    Wv_d = nc.dram_tensor("Wv_t", [T, KD, KD, P, P], f32r, kind="ExternalInput")
    W1_d = nc.dram_tensor("W1_t", [T, KD, KD, P, P], f32r, kind="ExternalInput")
    W2_d = nc.dram_tensor("W2_t", [T, KD, KD, P, P], f32r, kind="ExternalInput")
    Wri_d = nc.dram_tensor("Wri_t", [T, KF, KD, P, P], f32r, kind="ExternalInput")
    Wgi_d = nc.dram_tensor("Wgi_t", [T, KF, KD, P, P], f32r, kind="ExternalInput")
    Wrs_d = nc.dram_tensor("Wrs_t", [T, KF, KD, P, P], f32r, kind="ExternalInput")
    Wgs_d = nc.dram_tensor("Wgs_t", [T, KF, KD, P, P], f32r, kind="ExternalInput")
    bu_d = nc.dram_tensor("bu_c", [T, P, KD], f32, kind="ExternalInput")
    bv_d = nc.dram_tensor("bv_c", [T, P, KD], f32, kind="ExternalInput")
    b1_d = nc.dram_tensor("b1_c", [T, P, KD], f32, kind="ExternalInput")
    b2_d = nc.dram_tensor("b2_c", [T, P, KD], f32, kind="ExternalInput")
    out_d = nc.dram_tensor("ZT", [D, C], f32, kind="ExternalOutput")

    def ff(ap):
        """read an f32r tile as plain f32 (zero-cost bitcast) for DVE/ACT/
        transpose consumption."""
        return ap.bitcast(f32)

    with tile.TileContext(nc) as tc:
        with (
            tc.tile_pool(name="const", bufs=1) as const,
            tc.tile_pool(name="blk", bufs=44) as blk,       # [128, C] transients
            tc.tile_pool(name="q96", bufs=22) as q96,       # [128, Q] transients
            tc.tile_pool(name="row", bufs=2) as row,        # [Q or 1, C]
            tc.tile_pool(name="stat", bufs=24) as stat,     # [p, 1]
            tc.tile_pool(name="wt", bufs=16) as wtp,        # weight stream
            tc.tile_pool(name="bias", bufs=4) as biasp,
            tc.tile_pool(name="acc", bufs=6, space="PSUM") as acc,
            tc.tile_pool(name="ptr", bufs=2, space="PSUM") as ptr,
        ):
            # ---------------- constants ----------------
            V_sb = const.tile([Q, D], f32r, name="V_sb")
            nc.sync.dma_start(V_sb, V_d[:, :])
            cm_bc = const.tile([P, C], f32, name="cm_bc_sb")
            nc.sync.dma_start(cm_bc, cmbc_d[:, :])
            qm_bc = const.tile([P, Q], f32, name="qm_bc_sb")
            nc.sync.dma_start(qm_bc, qmbc_d[:, :])
            cm_cols = const.tile([P, KC], f32, name="cm_cols_sb")
            nc.sync.dma_start(cm_cols, cmcol_d[:, :])
            ident = const.tile([P, P], f32, name="ident_sb")
            nc.sync.dma_start(ident, ident_d[:, :])
            diagm = const.tile([P, P], f32, name="diagm_sb")
            nc.sync.dma_start(diagm, diagm_d[:, :])
            ones_col = const.tile([P, 1], f32, name="ones_col_sb")
            nc.sync.dma_start(ones_col, ones_d[:, :])
            ones_lhs = const.tile([1, P], f32, name="ones_lhs_sb")
            nc.vector.memset(ones_lhs, 1.0)

            # V^T blocks (d on partitions), f32r for the QtT matmul rhs
            VT = []
            for k in range(KD):
                pt = ptr.tile([P, Q], f32, name="pt", tag="tr")
                nc.tensor.transpose(pt, ff(V_sb)[:, k * P:(k + 1) * P],
                                    ident[:Q, :Q])
                vt = const.tile([P, Q], f32r, name=f"VT{k}")
                nc.vector.tensor_copy(vt, pt)
                VT.append(vt)

            # xT state blocks (U^T)
            xT = []
            for k in range(KD):
                t_ = blk.tile([P, C], f32r, name=f"xT0_{k}", tag="blk")
                nc.sync.dma_start(t_, UT_d[k * P:(k + 1) * P, :])
                xT.append(t_)

            ET_state = None
            Bst = None

            # ------------- helpers -------------
            def load_bias(bias_dram, t):
                b = biasp.tile([P, KD], f32, name="b", tag="bias")
                nc.sync.dma_start(b, bias_dram[t])
                return b

            def mm_wT(W_dram, t, X, bias_sb, act, kt, out_name):
                """KD f32r blocks [128, C] = act(W^T @ X + b)."""
                outs = []
                for m in range(KD):
                    o = blk.tile([P, C], f32r, name=f"{out_name}{m}", tag="blk")
                    for lo, w in CCH:
                        ps = acc.tile([P, w], f32, name="ps", tag="acc")
                        for k in range(kt):
                            wt = wtp.tile([P, P], f32r, name="wtile", tag="wt")
                            nc.sync.dma_start(wt, W_dram[t, k, m])
                            nc.tensor.matmul(ps, wt, X[k][:, lo:lo + w],
                                             start=(k == 0), stop=(k == kt - 1))
                        nc.scalar.activation(o[:, lo:lo + w], ps, act,
                                             bias=bias_sb[:, m:m + 1])
                    outs.append(o)
                return outs

            def softmax_free(src, p, L, mask_bc, nm, out_dt=f32):
                """rowwise masked softmax over the free dim; src/dst [p, L].
                src tiles are f32."""
                pool, tg = (blk, "blk") if L == C else (q96, "q96")
                outs = []
                for i, s in enumerate(src):
                    negmx = stat.tile([p, 1], f32, name="negmx", tag="stat")
                    nc.vector.reduce_max(negmx, s, axis=AX.X, negate=True)
                    ex = pool.tile([p, L], f32, name=f"{nm}e{i}", tag=tg)
                    nc.scalar.activation(ex, s, AF.Exp, bias=negmx)
                    pm = pool.tile([p, L], f32, name=f"{nm}p{i}", tag=tg)
                    nc.vector.tensor_mul(pm, ex, mask_bc[:p, :L])
                    ssum = stat.tile([p, 1], f32, name="ssum", tag="stat")
                    nc.vector.reduce_sum(ssum, pm, axis=AX.X)
                    rec = stat.tile([p, 1], f32, name="rec", tag="stat")
                    nc.vector.reciprocal(rec, ssum)
                    o = pool.tile([p, L], out_dt, name=f"{nm}o{i}", tag=tg)
                    nc.scalar.activation(o, pm, AF.Copy, scale=rec)
                    outs.append(o)
                return outs

            def softmax_part(src, gamma, nm):
                """masked softmax over the PARTITION dim across KC row-blocks
                [128, C] (f32) of a (C, C) matrix; cmask along partitions.
                No max-subtraction (|values| < 70, exp fits fp32).
                Column sums via plain-fp32 PE ones-matmul.
                Output blocks are f32r (feed matmuls)."""
                pms = []
                for k, s in enumerate(src):
                    ex = blk.tile([P, C], f32, name=f"{nm}e{k}", tag="blk")
                    nc.scalar.activation(ex, s, AF.Exp)
                    pm = blk.tile([P, C], f32, name=f"{nm}m{k}", tag="blk")
                    nc.vector.tensor_scalar_mul(pm, ex, cm_cols[:, k:k + 1])
                    pms.append(pm)
                rec = row.tile([1, C], f32, name=f"{nm}rec", tag="rec1", bufs=2)
                for lo, w in CCH:
                    ps = ptr.tile([1, w], f32, name="ps", tag="tr")
                    for k in range(KC):
                        nc.tensor.matmul(ps, ones_col, pms[k][:, lo:lo + w],
                                         start=(k == 0), stop=(k == KC - 1))
                    nc.vector.reciprocal(rec[:, lo:lo + w], ps)
                if gamma != 1.0:
                    rec2 = row.tile([1, C], f32, name=f"{nm}rec2", tag="rec1",
                                    bufs=2)
                    nc.scalar.mul(rec2, rec, float(gamma))
                    rec = rec2
                recbc = blk.tile([P, C], f32, name=f"{nm}rbc", tag="blk")
                for lo, w in CCH:
                    ps = ptr.tile([P, w], f32, name="ps", tag="tr")
                    nc.tensor.matmul(ps, ones_lhs, rec[:, lo:lo + w],
                                     start=True, stop=True)
                    nc.vector.tensor_copy(recbc[:, lo:lo + w], ps)
                outs = []
                for k in range(KC):
                    o = blk.tile([P, C], f32r, name=f"{nm}o{k}", tag="blk")
                    nc.vector.tensor_mul(o, pms[k], recbc)
                    outs.append(o)
                return outs

            def transpose_blocks(src, nm, src_f32r=False, out_dt=f32r):
                """(C, C) as KC blocks [128, C] -> transposed blocks.
                Transposes run in plain fp32 on the PE."""
                outs = []
                for m in range(KC):
                    o = blk.tile([P, C], out_dt, name=f"{nm}{m}", tag="blk")
                    for k in range(KC):
                        pt = ptr.tile([P, P], f32, name="pt", tag="tr")
                        s = ff(src[k]) if src_f32r else src[k]
                        nc.tensor.transpose(pt, s[:, m * P:(m + 1) * P], ident)
                        nc.vector.tensor_copy(o[:, k * P:(k + 1) * P], pt)
                    outs.append(o)
                return outs

            def sfu(xTb, fTb, Wr_dram, Wg_dram, t, nm):
                """h = g*(r - x) + x, with r=relu(m@Wr), g=sigmoid(m@Wg),
                m = [x, f, x*f] (folded).  Fused per output block so r/g/temps
                die immediately.  xTb/fTb are f32r; h blocks are f32r."""
                prod = []
                for k in range(KD):
                    pr = blk.tile([P, C], f32r, name=f"{nm}pr{k}", tag="blk")
                    nc.vector.tensor_mul(pr, ff(xTb[k]), ff(fTb[k]))
                    prod.append(pr)
                mT = list(xTb) + list(fTb) + prod
                hT = []
                for m in range(KD):
                    rm = blk.tile([P, C], f32, name=f"{nm}r{m}", tag="blk")
                    gm = blk.tile([P, C], f32, name=f"{nm}g{m}", tag="blk")
                    pss = [(acc.tile([P, w], f32, name="psr", tag="acc"),
                            acc.tile([P, w], f32, name="psg", tag="acc"), lo, w)
                           for lo, w in CCH]
                    for k in range(KF):
                        wr = wtp.tile([P, P], f32r, name="wtr", tag="wt")
                        nc.sync.dma_start(wr, Wr_dram[t, k, m])
                        wg = wtp.tile([P, P], f32r, name="wtg", tag="wt")
                        nc.sync.dma_start(wg, Wg_dram[t, k, m])
                        st, sp = (k == 0), (k == KF - 1)
                        for psr, psg, lo, w in pss:
                            nc.tensor.matmul(psr, wr, mT[k][:, lo:lo + w],
                                             start=st, stop=sp)
                            nc.tensor.matmul(psg, wg, mT[k][:, lo:lo + w],
                                             start=st, stop=sp)
                    for psr, psg, lo, w in pss:
                        nc.scalar.activation(rm[:, lo:lo + w], psr, AF.Relu)
                        nc.scalar.activation(gm[:, lo:lo + w], psg, AF.Sigmoid)
                    t1 = blk.tile([P, C], f32, name=f"{nm}t1_{m}", tag="blk")
                    nc.vector.tensor_sub(t1, rm, ff(xTb[m]))
                    t2 = blk.tile([P, C], f32, name=f"{nm}t2_{m}", tag="blk")
                    nc.vector.tensor_mul(t2, gm, t1)
                    h = blk.tile([P, C], f32r, name=f"{nm}h{m}", tag="blk")
                    nc.vector.tensor_add(h, t2, ff(xTb[m]))
                    hT.append(h)
                return hT

            def evac_diag0(dst, ps, m, lo, w):
                """PSUM->SBUF evac of B row-block m, zeroing the diagonal."""
                dlo, dhi = m * P, (m + 1) * P
                s, e = max(lo, dlo), min(lo + w, dhi)
                if s < e:
                    if lo < s:
                        nc.scalar.copy(dst[:, lo:s], ps[:, 0:s - lo])
                    nc.vector.tensor_mul(dst[:, s:e], ps[:, s - lo:e - lo],
                                         diagm[:, 0:e - s])
                    if e < lo + w:
                        nc.scalar.copy(dst[:, e:lo + w], ps[:, e - lo:w])
                else:
                    nc.scalar.copy(dst[:, lo:lo + w], ps)

            # ================= stage loop =================
            for t in range(T):
                gi, gs = float(gammas_i[t]), float(gammas_s[t])

                # ---- InterAlign ----
                if t > 0:
                    B2s = softmax_free(Bst, P, C, cm_bc, f"B2s{t}_")
                    B2sT = transpose_blocks(B2s, f"B2sT{t}_")
                    B1s = softmax_part(Bst, gs, f"B1s{t}_")
                    EsT = softmax_free([ET_state], Q, C, cm_bc, f"EsT{t}_")[0]
                    Es = []
                    for k in range(KC):
                        pt = ptr.tile([P, Q], f32, name="pt", tag="tr")
                        nc.tensor.transpose(pt, EsT[:, k * P:(k + 1) * P],
                                            ident[:Q, :Q])
                        e_ = q96.tile([P, Q], f32r, name=f"Es{k}", tag="q96")
                        nc.scalar.mul(e_, pt, gi)  # fold gamma_i
                        Es.append(e_)
                else:
                    B2sT = B1s = Es = None

                bu_sb = load_bias(bu_d, t)
                CtT = mm_wT(Wu_d, t, xT, bu_sb, AF.Relu, KD, f"CtT{t}_")

                bv_sb = load_bias(bv_d, t)
                QtT = []
                for m in range(KD):
                    o = q96.tile([P, Q], f32r, name=f"QtT{t}_{m}", tag="q96")
                    ps = acc.tile([P, Q], f32, name="ps", tag="acc")
                    for k in range(KD):
                        wt = wtp.tile([P, P], f32r, name="wtv", tag="wt")
                        nc.sync.dma_start(wt, Wv_d[t, k, m])
                        nc.tensor.matmul(ps, wt, VT[k],
                                         start=(k == 0), stop=(k == KD - 1))
                    nc.scalar.activation(o, ps, AF.Relu, bias=bv_sb[:, m:m + 1])
                    QtT.append(o)

                # E^T = Qt @ Ct^T (+ gi * Es^T @ Bs^T), one PSUM accumulation
                ET_new = row.tile([Q, C], f32, name=f"ET{t}", tag="ET", bufs=2)
                for lo, w in CCH:
                    ps = acc.tile([Q, w], f32, name="ps", tag="acc")
                    for k in range(KD):
                        nc.tensor.matmul(ps, QtT[k], CtT[k][:, lo:lo + w],
                                         start=(k == 0),
                                         stop=(t == 0 and k == KD - 1))
                    if t > 0:
                        for k in range(KC):
                            nc.tensor.matmul(ps, Es[k], B2sT[k][:, lo:lo + w],
                                             start=False, stop=(k == KC - 1))
                    nc.scalar.copy(ET_new[:, lo:lo + w], ps)
                ET_state = ET_new

                # Ett = masked softmax over q of E natural, back to [Q, C] f32r
                E_nat = []
                for k in range(KC):
                    pt = ptr.tile([P, Q], f32, name="pt", tag="tr")
                    nc.tensor.transpose(pt, ET_new[:, k * P:(k + 1) * P],
                                        ident[:Q, :Q])
                    e_ = q96.tile([P, Q], f32, name=f"Enat{k}", tag="q96")
                    nc.vector.tensor_copy(e_, pt)
                    E_nat.append(e_)
                Ett = softmax_free(E_nat, P, Q, qm_bc, f"Ett{t}_")
                EttT = row.tile([Q, C], f32r, name=f"EttT{t}", tag="EttT",
                                bufs=2)
                for k in range(KC):
                    pt = ptr.tile([Q, P], f32, name="pt", tag="tr")
                    nc.tensor.transpose(pt, Ett[k], ident)
                    nc.vector.tensor_copy(EttT[:, k * P:(k + 1) * P], pt)

                # qctx^T = V^T @ EttT  (f32r out for the SFU matmuls)
                fT = []
                for m in range(KD):
                    o = blk.tile([P, C], f32r, name=f"qctxT{t}_{m}", tag="blk")
                    for lo, w in CCH:
                        ps = acc.tile([P, w], f32, name="ps", tag="acc")
                        nc.tensor.matmul(ps, V_sb[:, m * P:(m + 1) * P],
                                         EttT[:, lo:lo + w],
                                         start=True, stop=True)
                        nc.scalar.copy(o[:, lo:lo + w], ps)
                    fT.append(o)

                hT = sfu(xT, fT, Wri_d, Wgi_d, t, f"si{t}_")

                # ---- SelfAlign ----
                b1_sb = load_bias(b1_d, t)
                H1T = mm_wT(W1_d, t, hT, b1_sb, AF.Relu, KD, f"H1T{t}_")
                b2_sb = load_bias(b2_d, t)
                H2T = mm_wT(W2_d, t, hT, b2_sb, AF.Relu, KD, f"H2T{t}_")

                if t == 0:
                    # B state natural = (H1 @ H2^T) * (1 - eye); BnT via PE
                    Bst_new = []
                    for m in range(KC):
                        o = blk.tile([P, C], f32, name=f"Bst{m}", tag="blk")
                        for lo, w in CCH:
                            ps = acc.tile([P, w], f32, name="ps", tag="acc")
                            for k in range(KD):
                                nc.tensor.matmul(
                                    ps, H1T[k][:, m * P:(m + 1) * P],
                                    H2T[k][:, lo:lo + w],
                                    start=(k == 0), stop=(k == KD - 1))
                            evac_diag0(o, ps, m, lo, w)
                        Bst_new.append(o)
                    Bst = Bst_new
                    BnT = transpose_blocks(Bst, f"BnT{t}_", out_dt=f32)
                else:
                    # last stage: only B^T needed
                    BnT = []
                    for m in range(KC):
                        o = blk.tile([P, C], f32, name=f"BnT{t}_{m}", tag="blk")
                        for lo, w in CCH:
                            ps = acc.tile([P, w], f32, name="ps", tag="acc")
                            for k in range(KD):
                                nc.tensor.matmul(
                                    ps, H2T[k][:, m * P:(m + 1) * P],
                                    H1T[k][:, lo:lo + w],
                                    start=(k == 0), stop=False)
                            for k in range(KC):
                                nc.tensor.matmul(
                                    ps, B1s[k][:, m * P:(m + 1) * P],
                                    B2sT[k][:, lo:lo + w],
                                    start=False, stop=(k == KC - 1))
                            evac_diag0(o, ps, m, lo, w)
                        BnT.append(o)

                BttT = softmax_part(BnT, 1.0, f"Btt{t}_")
                hnat = transpose_blocks(hT, f"hnat{t}_", src_f32r=True)

                # hctx^T: lhsT = h natural, rhs = Btt^T
                fT2 = []
                for m in range(KD):
                    o = blk.tile([P, C], f32r, name=f"hctxT{t}_{m}", tag="blk")
                    for lo, w in CCH:
                        ps = acc.tile([P, w], f32, name="ps", tag="acc")
                        for k in range(KC):
                            nc.tensor.matmul(
                                ps, hnat[k][:, m * P:(m + 1) * P],
                                BttT[k][:, lo:lo + w],
                                start=(k == 0), stop=(k == KC - 1))
                        nc.scalar.copy(o[:, lo:lo + w], ps)
                    fT2.append(o)

                ZT = sfu(hT, fT2, Wrs_d, Wgs_d, t, f"ss{t}_")

                if t == T - 1:
                    for k in range(KD):
                        nc.sync.dma_start(out_d[k * P:(k + 1) * P, :],
                                          ff(ZT[k]))
                else:
                    xT = ZT

    nc.compile()
    return nc


# ================================================================ host side
def _fold_w(W):
    """(4d, dout) -> (3d, dout): [x, f, x*f, x-f] -> [x, f, x*f] basis."""
    d = W.shape[0] // 4
    W64 = W.astype(np.float64)
    return np.concatenate(
        [W64[0:d] + W64[3 * d:], W64[d:2 * d] - W64[3 * d:], W64[2 * d:3 * d]],
        axis=0).astype(np.float32)


def _tile_w(W):
    """(K, M) -> (K/128, M/128, 128, 128) contiguous tiles."""
    K, M = W.shape
    return np.ascontiguousarray(
        W.reshape(K // P, P, M // P, P).transpose(0, 2, 1, 3))


def _prep_maps(inputs, C, D, Q, T):
    U = np.asarray(inputs['U'], dtype=np.float32)
    V = np.asarray(inputs['V'], dtype=np.float32)
    Um = np.asarray(inputs['U_mask'])
    Vm = np.asarray(inputs['V_mask'])
    nb = U.shape[0]
    KD = D // P
    KC = C // P

    shared = {
        'ident': np.eye(P, dtype=np.float32),
        'diagm': (1.0 - np.eye(P)).astype(np.float32),
        'ones_cr': np.ones((P, 1), np.float32),
    }
    for nm, key, fold in (('Wu_t', 'Wu', 0), ('Wv_t', 'Wv', 0),
                          ('W1_t', 'W1', 0), ('W2_t', 'W2', 0),
                          ('Wri_t', 'Wr_i', 1), ('Wgi_t', 'Wg_i', 1),
                          ('Wrs_t', 'Wr_s', 1), ('Wgs_t', 'Wg_s', 1)):
        W = np.asarray(inputs[key], dtype=np.float32)
        shared[nm] = np.ascontiguousarray(
            np.stack([_tile_w(_fold_w(W[t]) if fold else W[t])
                      for t in range(T)]))
    for nm, key in (('bu_c', 'bu'), ('bv_c', 'bv'), ('b1_c', 'b1'),
                    ('b2_c', 'b2')):
        b = np.asarray(inputs[key], dtype=np.float32)
        shared[nm] = np.ascontiguousarray(
            b.reshape(T, KD, P).transpose(0, 2, 1))

    maps = []
    for i in range(nb):
        m = dict(shared)
        m['UT'] = np.ascontiguousarray(U[i].T)
        m['Vn'] = np.ascontiguousarray(V[i])
        cm = Um[i].astype(np.float32)
        qm = Vm[i].astype(np.float32)
        m['cm_bc'] = np.ascontiguousarray(np.broadcast_to(cm[None, :], (P, C)))
        m['qm_bc'] = np.ascontiguousarray(np.broadcast_to(qm[None, :], (P, Q)))
        m['cm_cols'] = np.ascontiguousarray(cm.reshape(KC, P).T)
        maps.append(m)
    return maps


_PROG_CACHE = {}


def run_traced(inputs, trace=False, **run_kwargs):
    """Run on hardware; returns (full_output, BassKernelResults)."""
    from concourse.bass_utils import run_bass_kernel_spmd

    U = np.asarray(inputs['U'])
    nb, C, D = U.shape
    Q = np.asarray(inputs['V']).shape[1]
    T = np.asarray(inputs['Wu']).shape[0]
    gi = tuple(float(g) for g in np.asarray(inputs['gamma_i']))
    gs = tuple(float(g) for g in np.asarray(inputs['gamma_s']))

    key = (C, D, Q, T, gi, gs)
    if key not in _PROG_CACHE:
        _PROG_CACHE[key] = build_program(C, D, Q, T, gi, gs)
    nc = _PROG_CACHE[key]

    maps = _prep_maps(inputs, C, D, Q, T)
    res = run_bass_kernel_spmd(nc, maps, core_ids=list(range(len(maps))),
                               trace=trace, **run_kwargs)
    out = np.stack([np.ascontiguousarray(r_['ZT'].T) for r_ in res.results])
    return out.astype(np.float32), res


def kernel(**inputs) -> np.ndarray:
    return run_traced(inputs, trace=False)[0]


if __name__ == '__main__':
    import reference
    inputs = {k: np.asarray(v) for k, v in reference.setup_inputs().items()}
    out = kernel(**inputs)
    print(out.shape, out.dtype)



# revision 8
# speedup vs baseline: 5.0816x; 5.0816x over previous
"""Trainium2 Bass kernel for nn_Model_11458972746263 (2-stage Aligner:
InterAlign + SelfAlign with SFU fusion blocks, carried E/B attention state).

Sharding: data-parallel over batch — 8 batch elements -> 8 NeuronCores, one
identical Bass program, per-core input maps.

Weights are NOT replicated host-side: the axon host->device tunnel is the
wall-clock bottleneck (~50-90 MB/s aggregate), so each core uploads a 1/8
flat shard of every weight tensor and the program AllGathers the full
weights core-side over NeuronLink into Internal DRAM before first use.
Masks ship as [1, L] rows and are broadcast to [128, L] on the PE.  The
output returns as bf16 (device->host is ~16 MB/s; bf16 rounding of the
final activations costs ~2e-3 max-rel, well under the 2e-2 gate).

Per-core dataflow (one batch element, PE matmuls in float32r, which streams at
1 cyc/row for moving dims >= 256, ~4x faster than plain fp32):
  canonical state is TRANSPOSED xT (d, c) so every weight matmul
  out^T = act(W^T @ xT + b) takes W as lhsT *as stored* and bias+activation is
  a fused per-partition ACT op on PSUM evacuation.

float32r plumbing: any tensor CONSUMED by an f32r matmul must be produced
with dtype float32r (the producer rounds on write; plain-f32-bitcast is
rejected by the BIR verifier).  DMA from an f32r DRAM tensor counts.  Engines
reading f32r tiles for non-matmul ops use a zero-cost bitcast back to f32.
walrus also only allows ONE sync wait on self-loading (fp32/f32r) matmuls —
_split_matmul_waits() moves surplus waits onto PE NoOps.

Host-side prep (inside kernel(), plain numpy):
  - U -> U^T per core;  SFU weights folded 4d->3d ([x, f, x*f] basis, exact
    reparametrization of [x, f, x*f, x-f] @ W);
  - weights retiled to contiguous [128,128] blocks for max-BW DMA;
  - masks cast to f32 in broadcast ([128,C]) and per-partition column layouts.
"""

import numpy as np

P = 128


def _split_matmul_waits(nc):
    """This walrus build caps sync waits per lowered instruction struct (the
    self-loading fp32/f32r matmul S3_LW takes only ONE; ACT structs are also
    limited). Move surplus waits of every compute-engine instruction onto
    NoOps inserted just before it on the same engine — engine program order
    makes that equivalent."""
    import concourse.mybir as mybir
    skip = (mybir.InstNoOp, mybir.InstEventSemaphore)
    if hasattr(mybir, "InstDrain"):
        skip = skip + (mybir.InstDrain,)
    n_split = 0
    for f in nc.m.functions:
        for b in f.blocks:
            insts = b.instructions
            if not any(len(i.sync_info.on_wait) > 1 for i in insts
                       if i.sync_info is not None):
                continue
            out = []
            for inst in insts:
                si = inst.sync_info
                if (si is not None and len(si.on_wait) > 1
                        and not isinstance(inst, skip)
                        and not isinstance(inst, mybir.InstDMACopy)):
                    waits = list(si.on_wait)
                    for j, w in enumerate(waits[:-1]):
                        nop = mybir.InstNoOp(
                            name=f"{inst.name}_wsplit{j}",
                            engine=inst.engine, ins=[], outs=[],
                            sync_info=mybir.SyncInfo(on_wait=[w],
                                                     on_update=[]))
                        out.append(nop)
                    inst.sync_info = mybir.SyncInfo(
                        on_wait=[waits[-1]], on_update=list(si.on_update))
                    n_split += 1
                out.append(inst)
            b.instructions = out
    return n_split


def _chunks(n, target=384):
    """Split a free dim into PSUM-bank-sized chunks (<=512 fp32)."""
    if n <= 512:
        return [(0, n)]
    assert n % target == 0
    return [(i * target, target) for i in range(n // target)]


# ================================================================ builder
def build_program(C=768, D=768, Q=96, T=2, gammas_i=(3.0, 3.0),
                  gammas_s=(3.0, 3.0)):
    import concourse.mybir as mybir
    import concourse.tile as tile
    from concourse import bacc

    f32 = mybir.dt.float32
    f32r = mybir.dt.float32r
    bf16 = mybir.dt.bfloat16
    AF = mybir.ActivationFunctionType
    AX = mybir.AxisListType
    OP = mybir.AluOpType

    KC = C // P
    KD = D // P
    KF = (3 * D) // P
    NCORES = 8
    assert C % P == 0 and D % P == 0 and Q <= P and C == D

    CCH = _chunks(C)

    nc = bacc.Bacc("TRN2", target_bir_lowering=False, debug=False,
                   enable_asserts=True, num_devices=NCORES)

    # ---------------- DRAM I/O (per-core tensors) ----------------
    # f32r inputs: anything DMA'd straight into matmul operands.
    # Weight tensors arrive as flat per-core 1/8 shards; the program
    # AllGathers them into full [T, K, KD, P, P] Internal DRAM tensors.
    NW = T * KD * KD * P * P
    NWF = T * KF * KD * P * P
    assert NW % (NCORES * 1024) == 0 and NWF % (NCORES * 1024) == 0
    UT_d = nc.dram_tensor("UT", [D, C], f32r, kind="ExternalInput")
    V_d = nc.dram_tensor("Vn", [Q, D], f32r, kind="ExternalInput")
    cmrow_d = nc.dram_tensor("cm_row", [1, C], f32, kind="ExternalInput")
    qmrow_d = nc.dram_tensor("qm_row", [1, Q], f32, kind="ExternalInput")
    cmcol_d = nc.dram_tensor("cm_cols", [P, KC], f32, kind="ExternalInput")
    ident_d = nc.dram_tensor("ident", [P, P], f32, kind="ExternalInput")
    diagm_d = nc.dram_tensor("diagm", [P, P], f32, kind="ExternalInput")
    ones_d = nc.dram_tensor("ones_cr", [P, 1], f32, kind="ExternalInput")
    Wsh_d = {}
    for nm in ("Wu", "Wv", "W1", "W2"):
        Wsh_d[nm] = nc.dram_tensor(f"{nm}_s", [NW // NCORES // 1024, 1024],
                                   f32r, kind="ExternalInput")
    for nm in ("Wri", "Wgi", "Wrs", "Wgs"):
        Wsh_d[nm] = nc.dram_tensor(f"{nm}_s", [NWF // NCORES // 1024, 1024],
                                   f32r, kind="ExternalInput")
    bu_d = nc.dram_tensor("bu_c", [T, P, KD], f32, kind="ExternalInput")
    bv_d = nc.dram_tensor("bv_c", [T, P, KD], f32, kind="ExternalInput")
    b1_d = nc.dram_tensor("b1_c", [T, P, KD], f32, kind="ExternalInput")
    b2_d = nc.dram_tensor("b2_c", [T, P, KD], f32, kind="ExternalInput")
    out_d = nc.dram_tensor("ZT", [D, C], bf16, kind="ExternalOutput")

    def ff(ap):
        """read an f32r tile as plain f32 (zero-cost bitcast) for DVE/ACT/
        transpose consumption."""
        return ap.bitcast(f32)

    with tile.TileContext(nc) as tc:
        with (
            tc.tile_pool(name="const", bufs=1) as const,
            tc.tile_pool(name="blk", bufs=44) as blk,       # [128, C] transients
            tc.tile_pool(name="q96", bufs=22) as q96,       # [128, Q] transients
            tc.tile_pool(name="row", bufs=2) as row,        # [Q or 1, C]
            tc.tile_pool(name="stat", bufs=24) as stat,     # [p, 1]
            tc.tile_pool(name="wt", bufs=16) as wtp,        # weight stream
            tc.tile_pool(name="bias", bufs=4) as biasp,
            tc.tile_pool(name="acc", bufs=6, space="PSUM") as acc,
            tc.tile_pool(name="ptr", bufs=2, space="PSUM") as ptr,
            tc.tile_pool(name="dramw", bufs=1, space="DRAM") as dramw,
        ):
            # ---------------- weight AllGather (shards -> full) ----------
            def gather_w(nm, KT):
                sh_d = Wsh_d[nm]
                bw = dramw.tile(list(sh_d.shape), f32r, name=f"b_{nm}",
                                tag=f"b_{nm}")
                nc.sync.dma_start(bw, sh_d[:, :])
                g = dramw.tile([T, KT, KD, P, P], f32r, name=f"g_{nm}",
                               tag=f"g_{nm}", addr_space="Shared")
                nc.gpsimd.collective_compute(
                    "AllGather", OP.bypass,
                    replica_groups=[list(range(NCORES))],
                    ins=[bw.opt()], outs=[g.opt()])
                return g

            # in first-use order so early consumers unblock first
            Wu_d = gather_w("Wu", KD)
            Wv_d = gather_w("Wv", KD)
            Wri_d = gather_w("Wri", KF)
            Wgi_d = gather_w("Wgi", KF)
            W1_d = gather_w("W1", KD)
            W2_d = gather_w("W2", KD)
            Wrs_d = gather_w("Wrs", KF)
            Wgs_d = gather_w("Wgs", KF)

            # ---------------- constants ----------------
            V_sb = const.tile([Q, D], f32r, name="V_sb")
            nc.sync.dma_start(V_sb, V_d[:, :])
            cm_row = const.tile([1, C], f32, name="cm_row_sb")
            nc.sync.dma_start(cm_row, cmrow_d[:, :])
            qm_row = const.tile([1, Q], f32, name="qm_row_sb")
            nc.sync.dma_start(qm_row, qmrow_d[:, :])
            cm_cols = const.tile([P, KC], f32, name="cm_cols_sb")
            nc.sync.dma_start(cm_cols, cmcol_d[:, :])
            ident = const.tile([P, P], f32, name="ident_sb")
            nc.sync.dma_start(ident, ident_d[:, :])
            diagm = const.tile([P, P], f32, name="diagm_sb")
            nc.sync.dma_start(diagm, diagm_d[:, :])
            ones_col = const.tile([P, 1], f32, name="ones_col_sb")
            nc.sync.dma_start(ones_col, ones_d[:, :])
            ones_lhs = const.tile([1, P], f32, name="ones_lhs_sb")
            nc.vector.memset(ones_lhs, 1.0)

            # broadcast masks [1, L] -> [P, L] on the PE (ones outer product)
            cm_bc = const.tile([P, C], f32, name="cm_bc_sb")
            for lo, w in CCH:
                ps = acc.tile([P, w], f32, name="ps", tag="acc")
                nc.tensor.matmul(ps, ones_lhs, cm_row[:, lo:lo + w],
                                 start=True, stop=True)
                nc.vector.tensor_copy(cm_bc[:, lo:lo + w], ps)
            qm_bc = const.tile([P, Q], f32, name="qm_bc_sb")
            ps_qm = acc.tile([P, Q], f32, name="ps_qm", tag="acc")
            nc.tensor.matmul(ps_qm, ones_lhs, qm_row[:, :], start=True,
                             stop=True)
            nc.vector.tensor_copy(qm_bc, ps_qm)

            # V^T blocks (d on partitions), f32r for the QtT matmul rhs
            VT = []
            for k in range(KD):
                pt = ptr.tile([P, Q], f32, name="pt", tag="tr")
                nc.tensor.transpose(pt, ff(V_sb)[:, k * P:(k + 1) * P],
                                    ident[:Q, :Q])
                vt = const.tile([P, Q], f32r, name=f"VT{k}")
                nc.vector.tensor_copy(vt, pt)
                VT.append(vt)

            # xT state blocks (U^T)
            xT = []
            for k in range(KD):
                t_ = blk.tile([P, C], f32r, name=f"xT0_{k}", tag="blk")
                nc.sync.dma_start(t_, UT_d[k * P:(k + 1) * P, :])
                xT.append(t_)

            ET_state = None
            Bst = None

            # ------------- helpers -------------
            def load_bias(bias_dram, t):
                b = biasp.tile([P, KD], f32, name="b", tag="bias")
                nc.sync.dma_start(b, bias_dram[t])
                return b

            def mm_wT(W_dram, t, X, bias_sb, act, kt, out_name):
                """KD f32r blocks [128, C] = act(W^T @ X + b)."""
                outs = []
                for m in range(KD):
                    o = blk.tile([P, C], f32r, name=f"{out_name}{m}", tag="blk")
                    for lo, w in CCH:
                        ps = acc.tile([P, w], f32, name="ps", tag="acc")
                        for k in range(kt):
                            wt = wtp.tile([P, P], f32r, name="wtile", tag="wt")
                            nc.sync.dma_start(wt, W_dram[t, k, m])
                            nc.tensor.matmul(ps, wt, X[k][:, lo:lo + w],
                                             start=(k == 0), stop=(k == kt - 1))
                        nc.scalar.activation(o[:, lo:lo + w], ps, act,
                                             bias=bias_sb[:, m:m + 1])
                    outs.append(o)
                return outs

            def softmax_free(src, p, L, mask_bc, nm, out_dt=f32):
                """rowwise masked softmax over the free dim; src/dst [p, L].
                src tiles are f32."""
                pool, tg = (blk, "blk") if L == C else (q96, "q96")
                outs = []
                for i, s in enumerate(src):
                    negmx = stat.tile([p, 1], f32, name="negmx", tag="stat")
                    nc.vector.reduce_max(negmx, s, axis=AX.X, negate=True)
                    ex = pool.tile([p, L], f32, name=f"{nm}e{i}", tag=tg)
                    nc.scalar.activation(ex, s, AF.Exp, bias=negmx)
                    pm = pool.tile([p, L], f32, name=f"{nm}p{i}", tag=tg)
                    nc.vector.tensor_mul(pm, ex, mask_bc[:p, :L])
                    ssum = stat.tile([p, 1], f32, name="ssum", tag="stat")
                    nc.vector.reduce_sum(ssum, pm, axis=AX.X)
                    rec = stat.tile([p, 1], f32, name="rec", tag="stat")
                    nc.vector.reciprocal(rec, ssum)
                    o = pool.tile([p, L], out_dt, name=f"{nm}o{i}", tag=tg)
                    nc.scalar.activation(o, pm, AF.Copy, scale=rec)
                    outs.append(o)
                return outs

            def softmax_part(src, gamma, nm):
                """masked softmax over the PARTITION dim across KC row-blocks
                [128, C] (f32) of a (C, C) matrix; cmask along partitions.
                No max-subtraction (|values| < 70, exp fits fp32).
                Column sums via plain-fp32 PE ones-matmul.
                Output blocks are f32r (feed matmuls)."""
                pms = []
                for k, s in enumerate(src):
                    ex = blk.tile([P, C], f32, name=f"{nm}e{k}", tag="blk")
                    nc.scalar.activation(ex, s, AF.Exp)
                    pm = blk.tile([P, C], f32, name=f"{nm}m{k}", tag="blk")
                    nc.vector.tensor_scalar_mul(pm, ex, cm_cols[:, k:k + 1])
                    pms.append(pm)
                rec = row.tile([1, C], f32, name=f"{nm}rec", tag="rec1", bufs=2)
                for lo, w in CCH:
                    ps = ptr.tile([1, w], f32, name="ps", tag="tr")
                    for k in range(KC):
                        nc.tensor.matmul(ps, ones_col, pms[k][:, lo:lo + w],
                                         start=(k == 0), stop=(k == KC - 1))
                    nc.vector.reciprocal(rec[:, lo:lo + w], ps)
                if gamma != 1.0:
                    rec2 = row.tile([1, C], f32, name=f"{nm}rec2", tag="rec1",
                                    bufs=2)
                    nc.scalar.mul(rec2, rec, float(gamma))
                    rec = rec2
                recbc = blk.tile([P, C], f32, name=f"{nm}rbc", tag="blk")
                for lo, w in CCH:
                    ps = ptr.tile([P, w], f32, name="ps", tag="tr")
                    nc.tensor.matmul(ps, ones_lhs, rec[:, lo:lo + w],
                                     start=True, stop=True)
                    nc.vector.tensor_copy(recbc[:, lo:lo + w], ps)
                outs = []
                for k in range(KC):
                    o = blk.tile([P, C], f32r, name=f"{nm}o{k}", tag="blk")
                    nc.vector.tensor_mul(o, pms[k], recbc)
                    outs.append(o)
                return outs

            def transpose_blocks(src, nm, src_f32r=False, out_dt=f32r):
                """(C, C) as KC blocks [128, C] -> transposed blocks.
                Transposes run in plain fp32 on the PE."""
                outs = []
                for m in range(KC):
                    o = blk.tile([P, C], out_dt, name=f"{nm}{m}", tag="blk")
                    for k in range(KC):
                        pt = ptr.tile([P, P], f32, name="pt", tag="tr")
                        s = ff(src[k]) if src_f32r else src[k]
                        nc.tensor.transpose(pt, s[:, m * P:(m + 1) * P], ident)
                        nc.vector.tensor_copy(o[:, k * P:(k + 1) * P], pt)
                    outs.append(o)
                return outs

            def sfu(xTb, fTb, Wr_dram, Wg_dram, t, nm):
                """h = g*(r - x) + x, with r=relu(m@Wr), g=sigmoid(m@Wg),
                m = [x, f, x*f] (folded).  Fused per output block so r/g/temps
                die immediately.  xTb/fTb are f32r; h blocks are f32r."""
                prod = []
                for k in range(KD):
                    pr = blk.tile([P, C], f32r, name=f"{nm}pr{k}", tag="blk")
                    nc.vector.tensor_mul(pr, ff(xTb[k]), ff(fTb[k]))
                    prod.append(pr)
                mT = list(xTb) + list(fTb) + prod
                hT = []
                for m in range(KD):
                    rm = blk.tile([P, C], f32, name=f"{nm}r{m}", tag="blk")
                    gm = blk.tile([P, C], f32, name=f"{nm}g{m}", tag="blk")
                    pss = [(acc.tile([P, w], f32, name="psr", tag="acc"),
                            acc.tile([P, w], f32, name="psg", tag="acc"), lo, w)
                           for lo, w in CCH]
                    for k in range(KF):
                        wr = wtp.tile([P, P], f32r, name="wtr", tag="wt")
                        nc.sync.dma_start(wr, Wr_dram[t, k, m])
                        wg = wtp.tile([P, P], f32r, name="wtg", tag="wt")
                        nc.sync.dma_start(wg, Wg_dram[t, k, m])
                        st, sp = (k == 0), (k == KF - 1)
                        for psr, psg, lo, w in pss:
                            nc.tensor.matmul(psr, wr, mT[k][:, lo:lo + w],
                                             start=st, stop=sp)
                            nc.tensor.matmul(psg, wg, mT[k][:, lo:lo + w],
                                             start=st, stop=sp)
                    for psr, psg, lo, w in pss:
                        nc.scalar.activation(rm[:, lo:lo + w], psr, AF.Relu)
                        nc.scalar.activation(gm[:, lo:lo + w], psg, AF.Sigmoid)
                    t1 = blk.tile([P, C], f32, name=f"{nm}t1_{m}", tag="blk")
                    nc.vector.tensor_sub(t1, rm, ff(xTb[m]))
                    t2 = blk.tile([P, C], f32, name=f"{nm}t2_{m}", tag="blk")
                    nc.vector.tensor_mul(t2, gm, t1)
                    h = blk.tile([P, C], f32r, name=f"{nm}h{m}", tag="blk")
                    nc.vector.tensor_add(h, t2, ff(xTb[m]))
                    hT.append(h)
                return hT

            def evac_diag0(dst, ps, m, lo, w):
                """PSUM->SBUF evac of B row-block m, zeroing the diagonal."""
                dlo, dhi = m * P, (m + 1) * P
                s, e = max(lo, dlo), min(lo + w, dhi)
                if s < e:
                    if lo < s:
                        nc.scalar.copy(dst[:, lo:s], ps[:, 0:s - lo])
                    nc.vector.tensor_mul(dst[:, s:e], ps[:, s - lo:e - lo],
                                         diagm[:, 0:e - s])
                    if e < lo + w:
                        nc.scalar.copy(dst[:, e:lo + w], ps[:, e - lo:w])
                else:
                    nc.scalar.copy(dst[:, lo:lo + w], ps)

            # ================= stage loop =================
            for t in range(T):
                gi, gs = float(gammas_i[t]), float(gammas_s[t])

                # ---- InterAlign ----
                if t > 0:
                    B2s = softmax_free(Bst, P, C, cm_bc, f"B2s{t}_")
                    B2sT = transpose_blocks(B2s, f"B2sT{t}_")
                    B1s = softmax_part(Bst, gs, f"B1s{t}_")
                    EsT = softmax_free([ET_state], Q, C, cm_bc, f"EsT{t}_")[0]
                    Es = []
                    for k in range(KC):
                        pt = ptr.tile([P, Q], f32, name="pt", tag="tr")
                        nc.tensor.transpose(pt, EsT[:, k * P:(k + 1) * P],
                                            ident[:Q, :Q])
                        e_ = q96.tile([P, Q], f32r, name=f"Es{k}", tag="q96")
                        nc.scalar.mul(e_, pt, gi)  # fold gamma_i
                        Es.append(e_)
                else:
                    B2sT = B1s = Es = None

                bu_sb = load_bias(bu_d, t)
                CtT = mm_wT(Wu_d, t, xT, bu_sb, AF.Relu, KD, f"CtT{t}_")

                bv_sb = load_bias(bv_d, t)
                QtT = []
                for m in range(KD):
                    o = q96.tile([P, Q], f32r, name=f"QtT{t}_{m}", tag="q96")
                    ps = acc.tile([P, Q], f32, name="ps", tag="acc")
                    for k in range(KD):
                        wt = wtp.tile([P, P], f32r, name="wtv", tag="wt")
                        nc.sync.dma_start(wt, Wv_d[t, k, m])
                        nc.tensor.matmul(ps, wt, VT[k],
                                         start=(k == 0), stop=(k == KD - 1))
                    nc.scalar.activation(o, ps, AF.Relu, bias=bv_sb[:, m:m + 1])
                    QtT.append(o)

                # E^T = Qt @ Ct^T (+ gi * Es^T @ Bs^T), one PSUM accumulation
                ET_new = row.tile([Q, C], f32, name=f"ET{t}", tag="ET", bufs=2)
                for lo, w in CCH:
                    ps = acc.tile([Q, w], f32, name="ps", tag="acc")
                    for k in range(KD):
                        nc.tensor.matmul(ps, QtT[k], CtT[k][:, lo:lo + w],
                                         start=(k == 0),
                                         stop=(t == 0 and k == KD - 1))
                    if t > 0:
                        for k in range(KC):
                            nc.tensor.matmul(ps, Es[k], B2sT[k][:, lo:lo + w],
                                             start=False, stop=(k == KC - 1))
                    nc.scalar.copy(ET_new[:, lo:lo + w], ps)
                ET_state = ET_new

                # Ett = masked softmax over q of E natural, back to [Q, C] f32r
                E_nat = []
                for k in range(KC):
                    pt = ptr.tile([P, Q], f32, name="pt", tag="tr")
                    nc.tensor.transpose(pt, ET_new[:, k * P:(k + 1) * P],
                                        ident[:Q, :Q])
                    e_ = q96.tile([P, Q], f32, name=f"Enat{k}", tag="q96")
                    nc.vector.tensor_copy(e_, pt)
                    E_nat.append(e_)
                Ett = softmax_free(E_nat, P, Q, qm_bc, f"Ett{t}_")
                EttT = row.tile([Q, C], f32r, name=f"EttT{t}", tag="EttT",
                                bufs=2)
                for k in range(KC):
                    pt = ptr.tile([Q, P], f32, name="pt", tag="tr")
                    nc.tensor.transpose(pt, Ett[k], ident)
                    nc.vector.tensor_copy(EttT[:, k * P:(k + 1) * P], pt)

                # qctx^T = V^T @ EttT  (f32r out for the SFU matmuls)
                fT = []
                for m in range(KD):
                    o = blk.tile([P, C], f32r, name=f"qctxT{t}_{m}", tag="blk")
                    for lo, w in CCH:
                        ps = acc.tile([P, w], f32, name="ps", tag="acc")
                        nc.tensor.matmul(ps, V_sb[:, m * P:(m + 1) * P],
                                         EttT[:, lo:lo + w],
                                         start=True, stop=True)
                        nc.scalar.copy(o[:, lo:lo + w], ps)
                    fT.append(o)

                hT = sfu(xT, fT, Wri_d, Wgi_d, t, f"si{t}_")

                # ---- SelfAlign ----
                b1_sb = load_bias(b1_d, t)
                H1T = mm_wT(W1_d, t, hT, b1_sb, AF.Relu, KD, f"H1T{t}_")
                b2_sb = load_bias(b2_d, t)
                H2T = mm_wT(W2_d, t, hT, b2_sb, AF.Relu, KD, f"H2T{t}_")

                if t == 0:
                    # B state natural = (H1 @ H2^T) * (1 - eye); BnT via PE
                    Bst_new = []
                    for m in range(KC):
                        o = blk.tile([P, C], f32, name=f"Bst{m}", tag="blk")
                        for lo, w in CCH:
                            ps = acc.tile([P, w], f32, name="ps", tag="acc")
                            for k in range(KD):
                                nc.tensor.matmul(
                                    ps, H1T[k][:, m * P:(m + 1) * P],
                                    H2T[k][:, lo:lo + w],
                                    start=(k == 0), stop=(k == KD - 1))
                            evac_diag0(o, ps, m, lo, w)
                        Bst_new.append(o)
                    Bst = Bst_new
                    BnT = transpose_blocks(Bst, f"BnT{t}_", out_dt=f32)
                else:
                    # last stage: only B^T needed
                    BnT = []
                    for m in range(KC):
                        o = blk.tile([P, C], f32, name=f"BnT{t}_{m}", tag="blk")
                        for lo, w in CCH:
                            ps = acc.tile([P, w], f32, name="ps", tag="acc")
                            for k in range(KD):
                                nc.tensor.matmul(
                                    ps, H2T[k][:, m * P:(m + 1) * P],
                                    H1T[k][:, lo:lo + w],
                                    start=(k == 0), stop=False)
                            for k in range(KC):
                                nc.tensor.matmul(
                                    ps, B1s[k][:, m * P:(m + 1) * P],
                                    B2sT[k][:, lo:lo + w],
                                    start=False, stop=(k == KC - 1))
                            evac_diag0(o, ps, m, lo, w)
                        BnT.append(o)

                BttT = softmax_part(BnT, 1.0, f"Btt{t}_")
                hnat = transpose_blocks(hT, f"hnat{t}_", src_f32r=True)

                # hctx^T: lhsT = h natural, rhs = Btt^T
                fT2 = []
                for m in range(KD):
                    o = blk.tile([P, C], f32r, name=f"hctxT{t}_{m}", tag="blk")
                    for lo, w in CCH:
                        ps = acc.tile([P, w], f32, name="ps", tag="acc")
                        for k in range(KC):
                            nc.tensor.matmul(
                                ps, hnat[k][:, m * P:(m + 1) * P],
                                BttT[k][:, lo:lo + w],
                                start=(k == 0), stop=(k == KC - 1))
                        nc.scalar.copy(o[:, lo:lo + w], ps)
                    fT2.append(o)

                ZT = sfu(hT, fT2, Wrs_d, Wgs_d, t, f"ss{t}_")

                if t == T - 1:
                    for k in range(KD):
                        zb = blk.tile([P, C], bf16, name=f"Zb{k}", tag="blk")
                        nc.vector.tensor_copy(zb, ff(ZT[k]))
                        nc.sync.dma_start(out_d[k * P:(k + 1) * P, :], zb)
                else:
                    xT = ZT

    nc.compile()
    return nc


# ================================================================ host side
def _fold_w(W):
    """(4d, dout) -> (3d, dout): [x, f, x*f, x-f] -> [x, f, x*f] basis."""
    d = W.shape[0] // 4
    W64 = W.astype(np.float64)
    return np.concatenate(
        [W64[0:d] + W64[3 * d:], W64[d:2 * d] - W64[3 * d:], W64[2 * d:3 * d]],
        axis=0).astype(np.float32)


def _tile_w(W):
    """(K, M) -> (K/128, M/128, 128, 128) contiguous tiles."""
    K, M = W.shape
    return np.ascontiguousarray(
        W.reshape(K // P, P, M // P, P).transpose(0, 2, 1, 3))


def _prep_maps(inputs, C, D, Q, T):
    U = np.asarray(inputs['U'], dtype=np.float32)
    V = np.asarray(inputs['V'], dtype=np.float32)
    Um = np.asarray(inputs['U_mask'])
    Vm = np.asarray(inputs['V_mask'])
    nb = U.shape[0]
    KD = D // P
    KC = C // P
    NCORES = 8

    shared = {
        'ident': np.eye(P, dtype=np.float32),
        'diagm': (1.0 - np.eye(P)).astype(np.float32),
        'ones_cr': np.ones((P, 1), np.float32),
    }
    # weight shards: flat 1/8 of the tiled weight per core
    wshards = {}
    for nm, key, fold in (('Wu_s', 'Wu', 0), ('Wv_s', 'Wv', 0),
                          ('W1_s', 'W1', 0), ('W2_s', 'W2', 0),
                          ('Wri_s', 'Wr_i', 1), ('Wgi_s', 'Wg_i', 1),
                          ('Wrs_s', 'Wr_s', 1), ('Wgs_s', 'Wg_s', 1)):
        W = np.asarray(inputs[key], dtype=np.float32)
        Wt = np.ascontiguousarray(
            np.stack([_tile_w(_fold_w(W[t]) if fold else W[t])
                      for t in range(T)]))
        wshards[nm] = Wt.reshape(NCORES, -1, 1024)
    for nm, key in (('bu_c', 'bu'), ('bv_c', 'bv'), ('b1_c', 'b1'),
                    ('b2_c', 'b2')):
        b = np.asarray(inputs[key], dtype=np.float32)
        shared[nm] = np.ascontiguousarray(
            b.reshape(T, KD, P).transpose(0, 2, 1))

    maps = []
    for i in range(nb):
        m = dict(shared)
        for nm, sh in wshards.items():
            m[nm] = sh[i]
        m['UT'] = np.ascontiguousarray(U[i].T)
        m['Vn'] = np.ascontiguousarray(V[i])
        cm = Um[i].astype(np.float32)
        qm = Vm[i].astype(np.float32)
        m['cm_row'] = np.ascontiguousarray(cm[None, :])
        m['qm_row'] = np.ascontiguousarray(qm[None, :])
        m['cm_cols'] = np.ascontiguousarray(cm.reshape(KC, P).T)
        maps.append(m)
    return maps


_PROG_CACHE = {}


def run_traced(inputs, trace=False, **run_kwargs):
    """Run on hardware; returns (full_output, BassKernelResults)."""
    from concourse.bass_utils import run_bass_kernel_spmd

    U = np.asarray(inputs['U'])
    nb, C, D = U.shape
    Q = np.asarray(inputs['V']).shape[1]
    T = np.asarray(inputs['Wu']).shape[0]
    gi = tuple(float(g) for g in np.asarray(inputs['gamma_i']))
    gs = tuple(float(g) for g in np.asarray(inputs['gamma_s']))

    key = (C, D, Q, T, gi, gs)
    if key not in _PROG_CACHE:
        _PROG_CACHE[key] = build_program(C, D, Q, T, gi, gs)
    nc = _PROG_CACHE[key]

    maps = _prep_maps(inputs, C, D, Q, T)
    res = run_bass_kernel_spmd(nc, maps, core_ids=list(range(len(maps))),
                               trace=trace, **run_kwargs)
    out = np.stack([np.asarray(r_['ZT']).astype(np.float32).T
                    for r_ in res.results])
    return np.ascontiguousarray(out), res


def kernel(**inputs) -> np.ndarray:
    return run_traced(inputs, trace=False)[0]


if __name__ == '__main__':
    import reference
    inputs = {k: np.asarray(v) for k, v in reference.setup_inputs().items()}
    out = kernel(**inputs)
    print(out.shape, out.dtype)



# revision 16
# speedup vs baseline: 9.5896x; 1.8871x over previous
"""Trainium2 Bass kernel for nn_Model_11458972746263 (2-stage Aligner:
InterAlign + SelfAlign with SFU fusion blocks, carried E/B attention state).

Sharding: data-parallel over batch — 8 batch elements -> 8 NeuronCores, one
identical Bass program, per-core input maps.

Weights are NOT replicated host-side: the axon host->device tunnel is the
wall-clock bottleneck (~50-90 MB/s aggregate), so each core uploads a 1/8
flat shard of every weight tensor and the program AllGathers the full
weights core-side over NeuronLink into Internal DRAM before first use.
Masks ship as [1, L] rows and are broadcast to [128, L] on the PE.  The
output returns as bf16 (device->host is ~16 MB/s; bf16 rounding of the
final activations costs ~2e-3 max-rel, well under the 2e-2 gate).

Per-core dataflow (one batch element, PE matmuls in float32r, which streams at
1 cyc/row for moving dims >= 256, ~4x faster than plain fp32):
  canonical state is TRANSPOSED xT (d, c) so every weight matmul
  out^T = act(W^T @ xT + b) takes W as lhsT *as stored* and bias+activation is
  a fused per-partition ACT op on PSUM evacuation.

float32r plumbing: any tensor CONSUMED by an f32r matmul must be produced
with dtype float32r (the producer rounds on write; plain-f32-bitcast is
rejected by the BIR verifier).  DMA from an f32r DRAM tensor counts.  Engines
reading f32r tiles for non-matmul ops use a zero-cost bitcast back to f32.
walrus also only allows ONE sync wait on self-loading (fp32/f32r) matmuls —
_split_matmul_waits() moves surplus waits onto PE NoOps.

Host-side prep (inside kernel(), plain numpy):
  - U -> U^T per core;  SFU weights folded 4d->3d ([x, f, x*f] basis, exact
    reparametrization of [x, f, x*f, x-f] @ W);
  - weights retiled to contiguous [128,128] blocks for max-BW DMA;
  - masks cast to f32 in broadcast ([128,C]) and per-partition column layouts.
"""

import numpy as np

P = 128


def _enable_jax_compile_cache():
    """Persist XLA executables across processes: run_bass_via_pjrt builds a
    fresh jit per call, so without this every kernel() invocation pays the
    full XLA compile (~1.5s) even with a warm NEFF cache."""
    try:
        import jax
        jax.config.update("jax_compilation_cache_dir", "/tmp/jax_cache")
        jax.config.update("jax_persistent_cache_min_entry_size_bytes", -1)
        jax.config.update("jax_persistent_cache_min_compile_time_secs", 0.0)
    except Exception:
        pass


_enable_jax_compile_cache()


def _split_matmul_waits(nc):
    """This walrus build caps sync waits per lowered instruction struct (the
    self-loading fp32/f32r matmul S3_LW takes only ONE; ACT structs are also
    limited). Move surplus waits of every compute-engine instruction onto
    NoOps inserted just before it on the same engine — engine program order
    makes that equivalent."""
    import concourse.mybir as mybir
    skip = (mybir.InstNoOp, mybir.InstEventSemaphore)
    if hasattr(mybir, "InstDrain"):
        skip = skip + (mybir.InstDrain,)
    n_split = 0
    for f in nc.m.functions:
        for b in f.blocks:
            insts = b.instructions
            if not any(len(i.sync_info.on_wait) > 1 for i in insts
                       if i.sync_info is not None):
                continue
            out = []
            for inst in insts:
                si = inst.sync_info
                if (si is not None and len(si.on_wait) > 1
                        and not isinstance(inst, skip)
                        and not isinstance(inst, mybir.InstDMACopy)):
                    waits = list(si.on_wait)
                    for j, w in enumerate(waits[:-1]):
                        nop = mybir.InstNoOp(
                            name=f"{inst.name}_wsplit{j}",
                            engine=inst.engine, ins=[], outs=[],
                            sync_info=mybir.SyncInfo(on_wait=[w],
                                                     on_update=[]))
                        out.append(nop)
                    inst.sync_info = mybir.SyncInfo(
                        on_wait=[waits[-1]], on_update=list(si.on_update))
                    n_split += 1
                out.append(inst)
            b.instructions = out
    return n_split


def _chunks(n, target=384):
    """Split a free dim into PSUM-bank-sized chunks (<=512 fp32)."""
    if n <= 512:
        return [(0, n)]
    assert n % target == 0
    return [(i * target, target) for i in range(n // target)]


# ================================================================ builder
def build_program(C=768, D=768, Q=96, T=2, gammas_i=(3.0, 3.0),
                  gammas_s=(3.0, 3.0)):
    import concourse.mybir as mybir
    import concourse.tile as tile
    from concourse import bacc

    f32 = mybir.dt.float32
    f32r = mybir.dt.float32r
    bf16 = mybir.dt.bfloat16
    fp16 = mybir.dt.float16
    AF = mybir.ActivationFunctionType
    AX = mybir.AxisListType
    OP = mybir.AluOpType

    KC = C // P
    KD = D // P
    KF = (3 * D) // P
    NCORES = 8
    assert C % P == 0 and D % P == 0 and Q <= P and C == D

    CCH = _chunks(C)

    nc = bacc.Bacc("TRN2", target_bir_lowering=False, debug=False,
                   enable_asserts=True, num_devices=NCORES)

    # ---------------- DRAM I/O (per-core tensors) ----------------
    # f32r inputs: anything DMA'd straight into matmul operands.
    # Weight tensors arrive as flat per-core 1/8 shards; the program
    # AllGathers them into full [T, K, KD, P, P] Internal DRAM tensors.
    NW = T * KD * KD * P * P
    NWF = T * KF * KD * P * P
    assert NW % (NCORES * 1024) == 0 and NWF % (NCORES * 1024) == 0
    UT_d = nc.dram_tensor("UT", [D, C], fp16, kind="ExternalInput")
    V_d = nc.dram_tensor("Vn", [Q, D], fp16, kind="ExternalInput")
    cmrow_d = nc.dram_tensor("cm_row", [1, C], f32, kind="ExternalInput")
    qmrow_d = nc.dram_tensor("qm_row", [1, Q], f32, kind="ExternalInput")
    cmcol_d = nc.dram_tensor("cm_cols", [P, KC], f32, kind="ExternalInput")
    ident_d = nc.dram_tensor("ident", [P, P], f32, kind="ExternalInput")
    Wsh_d = {}
    for nm in ("Wu", "Wv", "W1", "W2"):
        Wsh_d[nm] = nc.dram_tensor(f"{nm}_s", [NW // NCORES // 1024, 1024],
                                   fp16, kind="ExternalInput")
    for nm in ("Wri", "Wgi", "Wrs", "Wgs"):
        Wsh_d[nm] = nc.dram_tensor(f"{nm}_s", [NWF // NCORES // 1024, 1024],
                                   fp16, kind="ExternalInput")
    bu_d = nc.dram_tensor("bu_c", [T, P, KD], f32, kind="ExternalInput")
    bv_d = nc.dram_tensor("bv_c", [T, P, KD], f32, kind="ExternalInput")
    b1_d = nc.dram_tensor("b1_c", [T, P, KD], f32, kind="ExternalInput")
    b2_d = nc.dram_tensor("b2_c", [T, P, KD], f32, kind="ExternalInput")
    out_d = nc.dram_tensor("ZT", [D, C], bf16, kind="ExternalOutput")

    def ff(ap):
        """read an f32r tile as plain f32 (zero-cost bitcast) for DVE/ACT/
        transpose consumption."""
        return ap.bitcast(f32)

    with tile.TileContext(nc) as tc:
        with (
            tc.tile_pool(name="const", bufs=1) as const,
            tc.tile_pool(name="blk", bufs=44) as blk,       # [128, C] transients
            tc.tile_pool(name="q96", bufs=22) as q96,       # [128, Q] transients
            tc.tile_pool(name="row", bufs=2) as row,        # [Q or 1, C]
            tc.tile_pool(name="stat", bufs=24) as stat,     # [p, 1]
            tc.tile_pool(name="wt", bufs=16) as wtp,        # weight stream
            tc.tile_pool(name="bias", bufs=4) as biasp,
            tc.tile_pool(name="acc", bufs=6, space="PSUM") as acc,
            tc.tile_pool(name="ptr", bufs=2, space="PSUM") as ptr,
            tc.tile_pool(name="dramw", bufs=1, space="DRAM") as dramw,
        ):
            # ---------------- weight AllGather (fp16 shards -> full) -----
            def gather_w(nm, KT):
                sh_d = Wsh_d[nm]
                bw = dramw.tile(list(sh_d.shape), fp16, name=f"b_{nm}",
                                tag=f"b_{nm}")
                nc.sync.dma_start(bw, sh_d[:, :])
                g16 = dramw.tile([T, KT, KD, P, P], fp16, name=f"g16_{nm}",
                                 tag=f"g16_{nm}", addr_space="Shared")
                nc.gpsimd.collective_compute(
                    "AllGather", OP.bypass,
                    replica_groups=[list(range(NCORES))],
                    ins=[bw.opt()], outs=[g16.opt()])
                return g16

            def convert_w(g16, nm, KT):
                """fp16 gathered weights -> f32r, one SBUF pass per (t, k)
                row of [P, P] tiles.  The DMA in/out use identical AP pairs
                so the elementwise correspondence is preserved regardless of
                descriptor layout."""
                g = dramw.tile([T, KT, KD, P, P], f32r, name=f"g_{nm}",
                               tag=f"g_{nm}")
                for t_ in range(T):
                    for k_ in range(KT):
                        s16 = blk.tile([P, KD * P], fp16, name=f"c16_{nm}",
                                       tag="blk")
                        nc.sync.dma_start(s16, g16[t_, k_])
                        s32 = blk.tile([P, KD * P], f32r, name=f"c32_{nm}",
                                       tag="blk")
                        nc.vector.tensor_copy(s32, s16)
                        nc.sync.dma_start(g[t_, k_], s32)
                return g

            # gathers first (in first-use order), conversions follow
            g16s = {nm: gather_w(nm, KT) for nm, KT in
                    (("Wu", KD), ("Wv", KD), ("Wri", KF), ("Wgi", KF),
                     ("W1", KD), ("W2", KD), ("Wrs", KF), ("Wgs", KF))}
            Wu_d = convert_w(g16s["Wu"], "Wu", KD)
            Wv_d = convert_w(g16s["Wv"], "Wv", KD)
            Wri_d = convert_w(g16s["Wri"], "Wri", KF)
            Wgi_d = convert_w(g16s["Wgi"], "Wgi", KF)
            W1_d = convert_w(g16s["W1"], "W1", KD)
            W2_d = convert_w(g16s["W2"], "W2", KD)
            Wrs_d = convert_w(g16s["Wrs"], "Wrs", KF)
            Wgs_d = convert_w(g16s["Wgs"], "Wgs", KF)

            # ---------------- constants ----------------
            v16 = blk.tile([Q, D], fp16, name="v16", tag="blk")
            nc.sync.dma_start(v16, V_d[:, :])
            V_sb = const.tile([Q, D], f32r, name="V_sb")
            nc.vector.tensor_copy(V_sb, v16)
            cm_row = const.tile([1, C], f32, name="cm_row_sb")
            nc.sync.dma_start(cm_row, cmrow_d[:, :])
            qm_row = const.tile([1, Q], f32, name="qm_row_sb")
            nc.sync.dma_start(qm_row, qmrow_d[:, :])
            cm_cols = const.tile([P, KC], f32, name="cm_cols_sb")
            nc.sync.dma_start(cm_cols, cmcol_d[:, :])
            ident = const.tile([P, P], f32, name="ident_sb")
            nc.sync.dma_start(ident, ident_d[:, :])
            ones_col = const.tile([P, 1], f32, name="ones_col_sb")
            nc.vector.memset(ones_col, 1.0)
            ones_lhs = const.tile([1, P], f32, name="ones_lhs_sb")
            nc.vector.memset(ones_lhs, 1.0)
            # diagm = 1 - eye, derived on device
            ones_pp = blk.tile([P, P], f32, name="ones_pp", tag="blk")
            nc.vector.memset(ones_pp, 1.0)
            diagm = const.tile([P, P], f32, name="diagm_sb")
            nc.vector.tensor_sub(diagm, ones_pp, ident)

            # broadcast masks [1, L] -> [P, L] on the PE (ones outer product)
            cm_bc = const.tile([P, C], f32, name="cm_bc_sb")
            for lo, w in CCH:
                ps = acc.tile([P, w], f32, name="ps", tag="acc")
                nc.tensor.matmul(ps, ones_lhs, cm_row[:, lo:lo + w],
                                 start=True, stop=True)
                nc.vector.tensor_copy(cm_bc[:, lo:lo + w], ps)
            qm_bc = const.tile([P, Q], f32, name="qm_bc_sb")
            ps_qm = acc.tile([P, Q], f32, name="ps_qm", tag="acc")
            nc.tensor.matmul(ps_qm, ones_lhs, qm_row[:, :], start=True,
                             stop=True)
            nc.vector.tensor_copy(qm_bc, ps_qm)

            # V^T blocks (d on partitions), f32r for the QtT matmul rhs
            VT = []
            for k in range(KD):
                pt = ptr.tile([P, Q], f32, name="pt", tag="tr")
                nc.tensor.transpose(pt, ff(V_sb)[:, k * P:(k + 1) * P],
                                    ident[:Q, :Q])
                vt = const.tile([P, Q], f32r, name=f"VT{k}")
                nc.vector.tensor_copy(vt, pt)
                VT.append(vt)

            # xT state blocks (U^T), fp16 upload -> f32r
            xT = []
            for k in range(KD):
                u16 = blk.tile([P, C], fp16, name=f"u16_{k}", tag="blk")
                nc.sync.dma_start(u16, UT_d[k * P:(k + 1) * P, :])
                t_ = blk.tile([P, C], f32r, name=f"xT0_{k}", tag="blk")
                nc.vector.tensor_copy(t_, u16)
                xT.append(t_)

            ET_state = None
            Bst = None

            # ------------- helpers -------------
            def load_bias(bias_dram, t):
                b = biasp.tile([P, KD], f32, name="b", tag="bias")
                nc.sync.dma_start(b, bias_dram[t])
                return b

            def mm_wT(W_dram, t, X, bias_sb, act, kt, out_name):
                """KD f32r blocks [128, C] = act(W^T @ X + b)."""
                outs = []
                for m in range(KD):
                    o = blk.tile([P, C], f32r, name=f"{out_name}{m}", tag="blk")
                    for lo, w in CCH:
                        ps = acc.tile([P, w], f32, name="ps", tag="acc")
                        for k in range(kt):
                            wt = wtp.tile([P, P], f32r, name="wtile", tag="wt")
                            nc.sync.dma_start(wt, W_dram[t, k, m])
                            nc.tensor.matmul(ps, wt, X[k][:, lo:lo + w],
                                             start=(k == 0), stop=(k == kt - 1))
                        nc.scalar.activation(o[:, lo:lo + w], ps, act,
                                             bias=bias_sb[:, m:m + 1])
                    outs.append(o)
                return outs

            def softmax_free(src, p, L, mask_bc, nm, out_dt=f32):
                """rowwise masked softmax over the free dim; src/dst [p, L].
                src tiles are f32."""
                pool, tg = (blk, "blk") if L == C else (q96, "q96")
                outs = []
                for i, s in enumerate(src):
                    negmx = stat.tile([p, 1], f32, name="negmx", tag="stat")
                    nc.vector.reduce_max(negmx, s, axis=AX.X, negate=True)
                    ex = pool.tile([p, L], f32, name=f"{nm}e{i}", tag=tg)
                    nc.scalar.activation(ex, s, AF.Exp, bias=negmx)
                    pm = pool.tile([p, L], f32, name=f"{nm}p{i}", tag=tg)
                    nc.vector.tensor_mul(pm, ex, mask_bc[:p, :L])
                    ssum = stat.tile([p, 1], f32, name="ssum", tag="stat")
                    nc.vector.reduce_sum(ssum, pm, axis=AX.X)
                    rec = stat.tile([p, 1], f32, name="rec", tag="stat")
                    nc.vector.reciprocal(rec, ssum)
                    o = pool.tile([p, L], out_dt, name=f"{nm}o{i}", tag=tg)
                    nc.scalar.activation(o, pm, AF.Copy, scale=rec)
                    outs.append(o)
                return outs

            def softmax_part(src, gamma, nm):
                """masked softmax over the PARTITION dim across KC row-blocks
                [128, C] (f32) of a (C, C) matrix; cmask along partitions.
                No max-subtraction (|values| < 70, exp fits fp32).
                Column sums via plain-fp32 PE ones-matmul.
                Output blocks are f32r (feed matmuls)."""
                pms = []
                for k, s in enumerate(src):
                    ex = blk.tile([P, C], f32, name=f"{nm}e{k}", tag="blk")
                    nc.scalar.activation(ex, s, AF.Exp)
                    pm = blk.tile([P, C], f32, name=f"{nm}m{k}", tag="blk")
                    nc.vector.tensor_scalar_mul(pm, ex, cm_cols[:, k:k + 1])
                    pms.append(pm)
                rec = row.tile([1, C], f32, name=f"{nm}rec", tag="rec1", bufs=2)
                for lo, w in CCH:
                    ps = ptr.tile([1, w], f32, name="ps", tag="tr")
                    for k in range(KC):
                        nc.tensor.matmul(ps, ones_col, pms[k][:, lo:lo + w],
                                         start=(k == 0), stop=(k == KC - 1))
                    nc.vector.reciprocal(rec[:, lo:lo + w], ps)
                if gamma != 1.0:
                    rec2 = row.tile([1, C], f32, name=f"{nm}rec2", tag="rec1",
                                    bufs=2)
                    nc.scalar.mul(rec2, rec, float(gamma))
                    rec = rec2
                recbc = blk.tile([P, C], f32, name=f"{nm}rbc", tag="blk")
                for lo, w in CCH:
                    ps = ptr.tile([P, w], f32, name="ps", tag="tr")
                    nc.tensor.matmul(ps, ones_lhs, rec[:, lo:lo + w],
                                     start=True, stop=True)
                    nc.vector.tensor_copy(recbc[:, lo:lo + w], ps)
                outs = []
                for k in range(KC):
                    o = blk.tile([P, C], f32r, name=f"{nm}o{k}", tag="blk")
                    nc.vector.tensor_mul(o, pms[k], recbc)
                    outs.append(o)
                return outs

            def transpose_blocks(src, nm, src_f32r=False, out_dt=f32r):
                """(C, C) as KC blocks [128, C] -> transposed blocks.
                Transposes run in plain fp32 on the PE."""
                outs = []
                for m in range(KC):
                    o = blk.tile([P, C], out_dt, name=f"{nm}{m}", tag="blk")
                    for k in range(KC):
                        pt = ptr.tile([P, P], f32, name="pt", tag="tr")
                        s = ff(src[k]) if src_f32r else src[k]
                        nc.tensor.transpose(pt, s[:, m * P:(m + 1) * P], ident)
                        nc.vector.tensor_copy(o[:, k * P:(k + 1) * P], pt)
                    outs.append(o)
                return outs

            def sfu(xTb, fTb, Wr_dram, Wg_dram, t, nm):
                """h = g*(r - x) + x, with r=relu(m@Wr), g=sigmoid(m@Wg),
                m = [x, f, x*f] (folded).  Fused per output block so r/g/temps
                die immediately.  xTb/fTb are f32r; h blocks are f32r."""
                prod = []
                for k in range(KD):
                    pr = blk.tile([P, C], f32r, name=f"{nm}pr{k}", tag="blk")
                    nc.vector.tensor_mul(pr, ff(xTb[k]), ff(fTb[k]))
                    prod.append(pr)
                mT = list(xTb) + list(fTb) + prod
                hT = []
                for m in range(KD):
                    rm = blk.tile([P, C], f32, name=f"{nm}r{m}", tag="blk")
                    gm = blk.tile([P, C], f32, name=f"{nm}g{m}", tag="blk")
                    pss = [(acc.tile([P, w], f32, name="psr", tag="acc"),
                            acc.tile([P, w], f32, name="psg", tag="acc"), lo, w)
                           for lo, w in CCH]
                    for k in range(KF):
                        wr = wtp.tile([P, P], f32r, name="wtr", tag="wt")
                        nc.sync.dma_start(wr, Wr_dram[t, k, m])
                        wg = wtp.tile([P, P], f32r, name="wtg", tag="wt")
                        nc.sync.dma_start(wg, Wg_dram[t, k, m])
                        st, sp = (k == 0), (k == KF - 1)
                        for psr, psg, lo, w in pss:
                            nc.tensor.matmul(psr, wr, mT[k][:, lo:lo + w],
                                             start=st, stop=sp)
                            nc.tensor.matmul(psg, wg, mT[k][:, lo:lo + w],
                                             start=st, stop=sp)
                    for psr, psg, lo, w in pss:
                        nc.scalar.activation(rm[:, lo:lo + w], psr, AF.Relu)
                        nc.scalar.activation(gm[:, lo:lo + w], psg, AF.Sigmoid)
                    t1 = blk.tile([P, C], f32, name=f"{nm}t1_{m}", tag="blk")
                    nc.vector.tensor_sub(t1, rm, ff(xTb[m]))
                    t2 = blk.tile([P, C], f32, name=f"{nm}t2_{m}", tag="blk")
                    nc.vector.tensor_mul(t2, gm, t1)
                    h = blk.tile([P, C], f32r, name=f"{nm}h{m}", tag="blk")
                    nc.vector.tensor_add(h, t2, ff(xTb[m]))
                    hT.append(h)
                return hT

            def evac_diag0(dst, ps, m, lo, w):
                """PSUM->SBUF evac of B row-block m, zeroing the diagonal."""
                dlo, dhi = m * P, (m + 1) * P
                s, e = max(lo, dlo), min(lo + w, dhi)
                if s < e:
                    if lo < s:
                        nc.scalar.copy(dst[:, lo:s], ps[:, 0:s - lo])
                    nc.vector.tensor_mul(dst[:, s:e], ps[:, s - lo:e - lo],
                                         diagm[:, 0:e - s])
                    if e < lo + w:
                        nc.scalar.copy(dst[:, e:lo + w], ps[:, e - lo:w])
                else:
                    nc.scalar.copy(dst[:, lo:lo + w], ps)

            # ================= stage loop =================
            for t in range(T):
                gi, gs = float(gammas_i[t]), float(gammas_s[t])

                # ---- InterAlign ----
                if t > 0:
                    B2s = softmax_free(Bst, P, C, cm_bc, f"B2s{t}_")
                    B2sT = transpose_blocks(B2s, f"B2sT{t}_")
                    B1s = softmax_part(Bst, gs, f"B1s{t}_")
                    EsT = softmax_free([ET_state], Q, C, cm_bc, f"EsT{t}_")[0]
                    Es = []
                    for k in range(KC):
                        pt = ptr.tile([P, Q], f32, name="pt", tag="tr")
                        nc.tensor.transpose(pt, EsT[:, k * P:(k + 1) * P],
                                            ident[:Q, :Q])
                        e_ = q96.tile([P, Q], f32r, name=f"Es{k}", tag="q96")
                        nc.scalar.mul(e_, pt, gi)  # fold gamma_i
                        Es.append(e_)
                else:
                    B2sT = B1s = Es = None

                bu_sb = load_bias(bu_d, t)
                CtT = mm_wT(Wu_d, t, xT, bu_sb, AF.Relu, KD, f"CtT{t}_")

                bv_sb = load_bias(bv_d, t)
                QtT = []
                for m in range(KD):
                    o = q96.tile([P, Q], f32r, name=f"QtT{t}_{m}", tag="q96")
                    ps = acc.tile([P, Q], f32, name="ps", tag="acc")
                    for k in range(KD):
                        wt = wtp.tile([P, P], f32r, name="wtv", tag="wt")
                        nc.sync.dma_start(wt, Wv_d[t, k, m])
                        nc.tensor.matmul(ps, wt, VT[k],
                                         start=(k == 0), stop=(k == KD - 1))
                    nc.scalar.activation(o, ps, AF.Relu, bias=bv_sb[:, m:m + 1])
                    QtT.append(o)

                # E^T = Qt @ Ct^T (+ gi * Es^T @ Bs^T), one PSUM accumulation
                ET_new = row.tile([Q, C], f32, name=f"ET{t}", tag="ET", bufs=2)
                for lo, w in CCH:
                    ps = acc.tile([Q, w], f32, name="ps", tag="acc")
                    for k in range(KD):
                        nc.tensor.matmul(ps, QtT[k], CtT[k][:, lo:lo + w],
                                         start=(k == 0),
                                         stop=(t == 0 and k == KD - 1))
                    if t > 0:
                        for k in range(KC):
                            nc.tensor.matmul(ps, Es[k], B2sT[k][:, lo:lo + w],
                                             start=False, stop=(k == KC - 1))
                    nc.scalar.copy(ET_new[:, lo:lo + w], ps)
                ET_state = ET_new

                # Ett = masked softmax over q of E natural, back to [Q, C] f32r
                E_nat = []
                for k in range(KC):
                    pt = ptr.tile([P, Q], f32, name="pt", tag="tr")
                    nc.tensor.transpose(pt, ET_new[:, k * P:(k + 1) * P],
                                        ident[:Q, :Q])
                    e_ = q96.tile([P, Q], f32, name=f"Enat{k}", tag="q96")
                    nc.vector.tensor_copy(e_, pt)
                    E_nat.append(e_)
                Ett = softmax_free(E_nat, P, Q, qm_bc, f"Ett{t}_")
                EttT = row.tile([Q, C], f32r, name=f"EttT{t}", tag="EttT",
                                bufs=2)
                for k in range(KC):
                    pt = ptr.tile([Q, P], f32, name="pt", tag="tr")
                    nc.tensor.transpose(pt, Ett[k], ident)
                    nc.vector.tensor_copy(EttT[:, k * P:(k + 1) * P], pt)

                # qctx^T = V^T @ EttT  (f32r out for the SFU matmuls)
                fT = []
                for m in range(KD):
                    o = blk.tile([P, C], f32r, name=f"qctxT{t}_{m}", tag="blk")
                    for lo, w in CCH:
                        ps = acc.tile([P, w], f32, name="ps", tag="acc")
                        nc.tensor.matmul(ps, V_sb[:, m * P:(m + 1) * P],
                                         EttT[:, lo:lo + w],
                                         start=True, stop=True)
                        nc.scalar.copy(o[:, lo:lo + w], ps)
                    fT.append(o)

                hT = sfu(xT, fT, Wri_d, Wgi_d, t, f"si{t}_")

                # ---- SelfAlign ----
                b1_sb = load_bias(b1_d, t)
                H1T = mm_wT(W1_d, t, hT, b1_sb, AF.Relu, KD, f"H1T{t}_")
                b2_sb = load_bias(b2_d, t)
                H2T = mm_wT(W2_d, t, hT, b2_sb, AF.Relu, KD, f"H2T{t}_")

                if t == 0:
                    # B state natural = (H1 @ H2^T) * (1 - eye); BnT via PE
                    Bst_new = []
                    for m in range(KC):
                        o = blk.tile([P, C], f32, name=f"Bst{m}", tag="blk")
                        for lo, w in CCH:
                            ps = acc.tile([P, w], f32, name="ps", tag="acc")
                            for k in range(KD):
                                nc.tensor.matmul(
                                    ps, H1T[k][:, m * P:(m + 1) * P],
                                    H2T[k][:, lo:lo + w],
                                    start=(k == 0), stop=(k == KD - 1))
                            evac_diag0(o, ps, m, lo, w)
                        Bst_new.append(o)
                    Bst = Bst_new
                    BnT = transpose_blocks(Bst, f"BnT{t}_", out_dt=f32)
                else:
                    # last stage: only B^T needed
                    BnT = []
                    for m in range(KC):
                        o = blk.tile([P, C], f32, name=f"BnT{t}_{m}", tag="blk")
                        for lo, w in CCH:
                            ps = acc.tile([P, w], f32, name="ps", tag="acc")
                            for k in range(KD):
                                nc.tensor.matmul(
                                    ps, H2T[k][:, m * P:(m + 1) * P],
                                    H1T[k][:, lo:lo + w],
                                    start=(k == 0), stop=False)
                            for k in range(KC):
                                nc.tensor.matmul(
                                    ps, B1s[k][:, m * P:(m + 1) * P],
                                    B2sT[k][:, lo:lo + w],
                                    start=False, stop=(k == KC - 1))
                            evac_diag0(o, ps, m, lo, w)
                        BnT.append(o)

                BttT = softmax_part(BnT, 1.0, f"Btt{t}_")
                hnat = transpose_blocks(hT, f"hnat{t}_", src_f32r=True)

                # hctx^T: lhsT = h natural, rhs = Btt^T
                fT2 = []
                for m in range(KD):
                    o = blk.tile([P, C], f32r, name=f"hctxT{t}_{m}", tag="blk")
                    for lo, w in CCH:
                        ps = acc.tile([P, w], f32, name="ps", tag="acc")
                        for k in range(KC):
                            nc.tensor.matmul(
                                ps, hnat[k][:, m * P:(m + 1) * P],
                                BttT[k][:, lo:lo + w],
                                start=(k == 0), stop=(k == KC - 1))
                        nc.scalar.copy(o[:, lo:lo + w], ps)
                    fT2.append(o)

                ZT = sfu(hT, fT2, Wrs_d, Wgs_d, t, f"ss{t}_")

                if t == T - 1:
                    for k in range(KD):
                        zb = blk.tile([P, C], bf16, name=f"Zb{k}", tag="blk")
                        nc.vector.tensor_copy(zb, ff(ZT[k]))
                        nc.sync.dma_start(out_d[k * P:(k + 1) * P, :], zb)
                else:
                    xT = ZT

    nc.compile()
    return nc


# ================================================================ host side
def _fold_w(W):
    """(4d, dout) -> (3d, dout): [x, f, x*f, x-f] -> [x, f, x*f] basis."""
    d = W.shape[0] // 4
    W64 = W.astype(np.float64)
    return np.concatenate(
        [W64[0:d] + W64[3 * d:], W64[d:2 * d] - W64[3 * d:], W64[2 * d:3 * d]],
        axis=0).astype(np.float32)


def _tile_w(W):
    """(K, M) -> (K/128, M/128, 128, 128) contiguous tiles."""
    K, M = W.shape
    return np.ascontiguousarray(
        W.reshape(K // P, P, M // P, P).transpose(0, 2, 1, 3))


def _prep_maps(inputs, C, D, Q, T):
    U = np.asarray(inputs['U'], dtype=np.float32)
    V = np.asarray(inputs['V'], dtype=np.float32)
    Um = np.asarray(inputs['U_mask'])
    Vm = np.asarray(inputs['V_mask'])
    nb = U.shape[0]
    KD = D // P
    KC = C // P
    NCORES = 8

    shared = {
        'ident': np.eye(P, dtype=np.float32),
    }
    # weight shards: flat 1/8 of the tiled weight per core, fp16 on the wire
    wshards = {}
    for nm, key, fold in (('Wu_s', 'Wu', 0), ('Wv_s', 'Wv', 0),
                          ('W1_s', 'W1', 0), ('W2_s', 'W2', 0),
                          ('Wri_s', 'Wr_i', 1), ('Wgi_s', 'Wg_i', 1),
                          ('Wrs_s', 'Wr_s', 1), ('Wgs_s', 'Wg_s', 1)):
        W = np.asarray(inputs[key], dtype=np.float32)
        Wt = np.stack([_tile_w(_fold_w(W[t]) if fold else W[t])
                       for t in range(T)]).astype(np.float16)
        wshards[nm] = np.ascontiguousarray(Wt).reshape(NCORES, -1, 1024)
    for nm, key in (('bu_c', 'bu'), ('bv_c', 'bv'), ('b1_c', 'b1'),
                    ('b2_c', 'b2')):
        b = np.asarray(inputs[key], dtype=np.float32)
        shared[nm] = np.ascontiguousarray(
            b.reshape(T, KD, P).transpose(0, 2, 1))

    maps = []
    for i in range(nb):
        m = dict(shared)
        for nm, sh in wshards.items():
            m[nm] = sh[i]
        m['UT'] = np.ascontiguousarray(U[i].T.astype(np.float16))
        m['Vn'] = np.ascontiguousarray(V[i].astype(np.float16))
        cm = Um[i].astype(np.float32)
        qm = Vm[i].astype(np.float32)
        m['cm_row'] = np.ascontiguousarray(cm[None, :])
        m['qm_row'] = np.ascontiguousarray(qm[None, :])
        m['cm_cols'] = np.ascontiguousarray(cm.reshape(KC, P).T)
        maps.append(m)
    return maps


_PROG_CACHE = {}


def run_traced(inputs, trace=False, **run_kwargs):
    """Run on hardware; returns (full_output, BassKernelResults)."""
    from concourse.bass_utils import run_bass_kernel_spmd

    U = np.asarray(inputs['U'])
    nb, C, D = U.shape
    Q = np.asarray(inputs['V']).shape[1]
    T = np.asarray(inputs['Wu']).shape[0]
    gi = tuple(float(g) for g in np.asarray(inputs['gamma_i']))
    gs = tuple(float(g) for g in np.asarray(inputs['gamma_s']))

    key = (C, D, Q, T, gi, gs)
    if key not in _PROG_CACHE:
        _PROG_CACHE[key] = build_program(C, D, Q, T, gi, gs)
    nc = _PROG_CACHE[key]

    maps = _prep_maps(inputs, C, D, Q, T)
    res = run_bass_kernel_spmd(nc, maps, core_ids=list(range(len(maps))),
                               trace=trace, **run_kwargs)
    out = np.stack([np.asarray(r_['ZT']).astype(np.float32).T
                    for r_ in res.results])
    return np.ascontiguousarray(out), res


def kernel(**inputs) -> np.ndarray:
    return run_traced(inputs, trace=False)[0]


if __name__ == '__main__':
    import reference
    inputs = {k: np.asarray(v) for k, v in reference.setup_inputs().items()}
    out = kernel(**inputs)
    print(out.shape, out.dtype)

